# revision 1
# baseline (speedup 1.0000x reference)
"""Trainium2 Bass kernel: cross-attention head scores with partial RoPE + padding mask.

Computes attn[b,s,e] = rope(x_audio[b] @ W_q.T + b_q)[s] . rope(x_event[b] @ W_k.T + b_k)[e]
with masked (padding) event columns set to -inf.  Shapes: B=4, Sa=4096, Se=2048,
d=1024, d_h=1024, rot_dim=512.

Sharding (8 NeuronCores): core c -> batch b = c//2, audio-row half j = c%2
(2048 query rows).  Each core computes its full [2048, 2048] score block; the
K projection is recomputed by both cores of a batch (15% extra FLOPs, no
collectives).

Per-core pipeline (all tensors head-major, i.e. Q^T/K^T as [d_h, seq]):
  1. K^T = W_k.T-proj of x_event^T on the PE (float32r, full speed), bias +
     partial rope applied on ACT/DVE during PSUM evacuation; kept in SBUF.
  2. Per 512-column chunk of Q rows: same projection + rope, then the score
     GEMM  scores[s,e] = sum_h Q^T[h,s] K^T[h,e]  on the PE, additive -inf
     mask fused into the PSUM->SBUF evacuation, DMA to DRAM.

RoPE in head-major layout: channel pairs (2i, 2i+1) live on adjacent SBUF
partitions, so rotate_half is a partition pair-swap (DVE stream_shuffle) and
the sign is folded into the sin table (host-prepared, rows 2i negated).
"""

import numpy as np

import concourse.bacc as bacc
import concourse.mybir as mybir
from concourse.tile import TileContext
from concourse.bass_utils import run_bass_kernel_spmd

# Problem constants (hardcoded per contest contract).
B, SA_FULL, SE, D, H = 4, 4096, 2048, 1024, 1024
ROT = H // 2            # 512 rotated channels
SA = SA_FULL // 2       # 2048 query rows per core
KC = D // 128           # 8 contraction chunks
HC = H // 128           # 8 head tiles
RC = ROT // 128         # 4 rotated head tiles
NB = 512                # free-dim chunk (= one fp32 PSUM bank)
NCH = SE // NB          # 4 column chunks
N_CORES = 8

# Partition pair-swap mask for stream_shuffle (32-partition groups).
_SWAP = [x for i in range(16) for x in (2 * i + 1, 2 * i)]

_MODULE_CACHE = None
LAST_RESULT = None


def _build_module():
    f32, f32r = mybir.dt.float32, mybir.dt.float32r
    AF = mybir.ActivationFunctionType

    nc = bacc.Bacc(trn_type="TRN2", name="xattn_head")

    xa = nc.dram_tensor("xa", [D, SA], f32r, kind="ExternalInput")   # x_audio slice, transposed
    xe = nc.dram_tensor("xe", [D, SE], f32r, kind="ExternalInput")   # x_event, transposed
    wq = nc.dram_tensor("wq", [D, H], f32r, kind="ExternalInput")    # W_q.T
    wk = nc.dram_tensor("wk", [D, H], f32r, kind="ExternalInput")    # W_k.T
    bq = nc.dram_tensor("bq", [128, HC], f32, kind="ExternalInput")  # b_q as [128, 8]
    bk = nc.dram_tensor("bk", [128, HC], f32, kind="ExternalInput")
    cq = nc.dram_tensor("cq", [ROT, SA], f32, kind="ExternalInput")  # cos, head-major
    sq = nc.dram_tensor("sq", [ROT, SA], f32, kind="ExternalInput")  # sin, sign-folded
    ck = nc.dram_tensor("ck", [ROT, SE], f32, kind="ExternalInput")
    sk = nc.dram_tensor("sk", [ROT, SE], f32, kind="ExternalInput")
    mk = nc.dram_tensor("mk", [128, SE], f32, kind="ExternalInput")  # additive mask rows
    out = nc.dram_tensor("out", [SA, SE], f32, kind="ExternalOutput")

    with TileContext(nc) as tc:
        with (
            tc.tile_pool(name="const", bufs=1) as wp,
            tc.tile_pool(name="ktp", bufs=1) as kp,
            tc.tile_pool(name="qtp", bufs=1) as qp,
            tc.tile_pool(name="xp", bufs=2) as xp,
            tc.tile_pool(name="tabp", bufs=1) as tp,
            tc.tile_pool(name="tmpp", bufs=2) as mp,
            tc.tile_pool(name="oevp", bufs=4) as op_,
            tc.tile_pool(name="pjp", bufs=4, space="PSUM") as pp,
            tc.tile_pool(name="scp", bufs=4, space="PSUM") as sp,
        ):
            w_sb = wp.tile([128, KC * H], f32r, name="w_sb")
            bq_sb = wp.tile([128, HC], f32, name="bq_sb")
            bk_sb = wp.tile([128, HC], f32, name="bk_sb")
            mk_sb = wp.tile([128, SE], f32, name="mk_sb")
            nc.sync.dma_start(
                out=w_sb[:, :].rearrange("p (kc h) -> p kc h", kc=KC),
                in_=wk.rearrange("(kc p) h -> p kc h", p=128))
            nc.sync.dma_start(out=bq_sb[:, :], in_=bq[:, :])
            nc.sync.dma_start(out=bk_sb[:, :], in_=bk[:, :])
            nc.sync.dma_start(out=mk_sb[:, :], in_=mk[:, :])

            # K^T resident in SBUF for the whole kernel: 8 tiles [128, 2048] f32r.
            kt = [kp.tile([128, SE], f32r, name=f"kt{h}", tag=f"kt{h}")
                  for h in range(HC)]

            def load_x_chunk(src, col, tag):
                x_sl = xp.tile([128, KC * NB], f32r, name="x_sl", tag=tag)
                nc.sync.dma_start(
                    out=x_sl[:, :].rearrange("p (kc n) -> p kc n", kc=KC),
                    in_=src[:, col:col + NB].rearrange("(kc p) n -> p kc n", p=128))
                return x_sl

            def load_tab(src, col, tag):
                t = tp.tile([128, RC * NB], f32, name="tab", tag=tag)
                nc.sync.dma_start(
                    out=t[:, :].rearrange("p (hc n) -> p hc n", hc=RC),
                    in_=src[:, col:col + NB].rearrange("(hc p) n -> p hc n", p=128))
                return t

            def proj_chunk(x_sl, cos_t, sin_t, b_sb, dests):
                """One 512-column chunk of a projection: 8 head tiles, bias +
                partial rope fused into PSUM evacuation.  dests[m] = [128, NB] AP."""
                for m in range(HC):
                    ps = pp.tile([128, NB], f32, name="ps", tag="ps")
                    for k in range(KC):
                        nc.tensor.matmul(
                            ps[:, :],
                            w_sb[:, k * H + m * 128: k * H + (m + 1) * 128],
                            x_sl[:, k * NB:(k + 1) * NB],
                            start=(k == 0), stop=(k == KC - 1))
                    if m < RC:
                        t0 = mp.tile([128, NB], f32, name="t0", tag="t0")
                        nc.scalar.activation(t0[:, :], ps[:, :], AF.Identity,
                                             bias=b_sb[:, m:m + 1])
                        tsh = mp.tile([128, NB], f32, name="tsh", tag="tsh")
                        nc.vector.stream_shuffle(tsh[:, :], t0[:, :], _SWAP)
                        t1 = mp.tile([128, NB], f32, name="t1", tag="t1")
                        nc.vector.tensor_mul(t1[:, :], t0[:, :],
                                             cos_t[:, m * NB:(m + 1) * NB])
                        t2 = mp.tile([128, NB], f32, name="t2", tag="t2")
                        nc.vector.tensor_mul(t2[:, :], tsh[:, :],
                                             sin_t[:, m * NB:(m + 1) * NB])
                        nc.vector.tensor_add(dests[m], t1[:, :], t2[:, :])
                    else:
                        nc.scalar.activation(dests[m], ps[:, :], AF.Identity,
                                             bias=b_sb[:, m:m + 1])

            # ---- Phase K: project + rope x_event into kt tiles ----
            for ec in range(NCH):
                xe_sl = load_x_chunk(xe, ec * NB, "xsl")
                ct = load_tab(ck, ec * NB, "ct")
                st = load_tab(sk, ec * NB, "st")
                proj_chunk(xe_sl, ct, st, bk_sb,
                           [kt[h][:, ec * NB:(ec + 1) * NB] for h in range(HC)])

            # ---- Swap weights to W_q.T (waits on last K-phase read of w_sb) ----
            nc.sync.dma_start(
                out=w_sb[:, :].rearrange("p (kc h) -> p kc h", kc=KC),
                in_=wq.rearrange("(kc p) h -> p kc h", p=128))

            # ---- Phase Q + scores, fused per 512-row chunk of queries ----
            for sc in range(NCH):
                xa_sl = load_x_chunk(xa, sc * NB, "xsl")
                cqt = load_tab(cq, sc * NB, "ct")
                sqt = load_tab(sq, sc * NB, "st")
                qts = [qp.tile([128, NB], f32r, name=f"qt{h}", tag=f"qt{h}")
                       for h in range(HC)]
                proj_chunk(xa_sl, cqt, sqt, bq_sb, [q[:, :] for q in qts])

                for sti in range(NB // 128):
                    for ec in range(NCH):
                        ps2 = sp.tile([128, NB], f32, name="ps2", tag="ps2")
                        for h in range(HC):
                            nc.tensor.matmul(
                                ps2[:, :],
                                qts[h][:, sti * 128:(sti + 1) * 128],
                                kt[h][:, ec * NB:(ec + 1) * NB],
                                start=(h == 0), stop=(h == HC - 1))
                        osb = op_.tile([128, NB], f32, name="osb", tag="osb")
                        nc.vector.tensor_add(osb[:, :], ps2[:, :],
                                             mk_sb[:, ec * NB:(ec + 1) * NB])
                        nc.sync.dma_start(
                            out=out[sc * NB + sti * 128: sc * NB + (sti + 1) * 128,
                                    ec * NB:(ec + 1) * NB],
                            in_=osb[:, :])

    nc.compile()
    return nc


def _get_module():
    global _MODULE_CACHE
    if _MODULE_CACHE is None:
        _MODULE_CACHE = _build_module()
    return _MODULE_CACHE


def kernel(x_audio, x_event, event_padding_mask, W_q, b_q, W_k, b_k):
    global LAST_RESULT
    x_audio = np.asarray(x_audio, dtype=np.float32)
    x_event = np.asarray(x_event, dtype=np.float32)
    event_padding_mask = np.asarray(event_padding_mask)
    W_q = np.asarray(W_q, dtype=np.float32)
    b_q = np.asarray(b_q, dtype=np.float32)
    W_k = np.asarray(W_k, dtype=np.float32)
    b_k = np.asarray(b_k, dtype=np.float32)

    # Rope tables, head-major, fp32 math matching the reference.
    inv_freq = (1.0 / (10000.0 ** (np.arange(0, ROT, 2, dtype=np.float32)
                                   / np.float32(ROT)))).astype(np.float32)
    pos = np.arange(SA_FULL, dtype=np.float32)
    freqs = pos[:, None] * inv_freq[None, :]                  # [Sa, 256]
    cos_hm = np.repeat(np.cos(freqs).T, 2, axis=0)            # [512, Sa]
    sin_hm = np.repeat(np.sin(freqs).T, 2, axis=0)
    sin_hm[0::2] *= -1.0                                      # fold rotate_half sign
    cos_hm = np.ascontiguousarray(cos_hm, dtype=np.float32)
    sin_hm = np.ascontiguousarray(sin_hm, dtype=np.float32)

    wqT = np.ascontiguousarray(W_q.T)
    wkT = np.ascontiguousarray(W_k.T)
    bq_sb = np.ascontiguousarray(b_q.reshape(HC, 128).T)
    bk_sb = np.ascontiguousarray(b_k.reshape(HC, 128).T)

    maskadd = np.where(event_padding_mask, np.float32(0.0),
                       np.float32(-np.inf)).astype(np.float32)  # [B, Se]
    mk_by_batch = [np.ascontiguousarray(np.broadcast_to(maskadd[b], (128, SE)))
                   for b in range(B)]
    xeT_by_batch = [np.ascontiguousarray(x_event[b].T) for b in range(B)]
    ck_hm = np.ascontiguousarray(cos_hm[:, :SE])
    sk_hm = np.ascontiguousarray(sin_hm[:, :SE])

    in_maps = []
    for c in range(N_CORES):
        b, j = divmod(c, 2)
        in_maps.append({
            "xa": np.ascontiguousarray(x_audio[b, j * SA:(j + 1) * SA].T),
            "xe": xeT_by_batch[b],
            "wq": wqT, "wk": wkT,
            "bq": bq_sb, "bk": bk_sb,
            "cq": np.ascontiguousarray(cos_hm[:, j * SA:(j + 1) * SA]),
            "sq": np.ascontiguousarray(sin_hm[:, j * SA:(j + 1) * SA]),
            "ck": ck_hm, "sk": sk_hm,
            "mk": mk_by_batch[b],
        })

    nc = _get_module()
    LAST_RESULT = run_bass_kernel_spmd(nc, in_maps, core_ids=list(range(N_CORES)))

    attn = np.empty((B, SA_FULL, SE), dtype=np.float32)
    for c in range(N_CORES):
        b, j = divmod(c, 2)
        attn[b, j * SA:(j + 1) * SA, :] = LAST_RESULT.results[c]["out"]
    return attn


# revision 8
# speedup vs baseline: 1.0329x; 1.0329x over previous
"""Trainium2 Bass kernel: cross-attention head scores with partial RoPE + padding mask.

Computes attn[b,s,e] = rope(x_audio[b] @ W_q.T + b_q)[s] . rope(x_event[b] @ W_k.T + b_k)[e]
with masked (padding) event columns set to -inf.  Shapes: B=4, Sa=4096, Se=2048,
d=1024, d_h=1024, rot_dim=512.

Sharding (8 NeuronCores): core c -> batch b = c//2, audio-row half j = c%2
(2048 query rows).  Each core computes its full [2048, 2048] score block; the
K projection is recomputed by both cores of a batch (15% extra FLOPs, no
collectives).

Per-core pipeline (all tensors head-major, i.e. Q^T/K^T as [d_h, seq]):
  1. K^T = W_k.T-proj of x_event^T on the PE (float32r, full speed), bias +
     partial rope applied on ACT/DVE during PSUM evacuation; kept in SBUF.
  2. Per 512-column chunk of Q rows: same projection + rope, then the score
     GEMM  scores[s,e] = sum_h Q^T[h,s] K^T[h,e]  on the PE, additive -inf
     mask fused into the PSUM->SBUF evacuation, DMA to DRAM.

RoPE in head-major layout: channel pairs (2i, 2i+1) live on adjacent SBUF
partitions, so rotate_half is a partition pair-swap (DVE stream_shuffle) and
the sign is folded into the sin table (host-prepared, rows 2i negated).
"""

import numpy as np

import concourse.bacc as bacc
import concourse.mybir as mybir
from concourse.tile import TileContext
from concourse.bass_utils import run_bass_kernel_spmd

# Problem constants (hardcoded per contest contract).
B, SA_FULL, SE, D, H = 4, 4096, 2048, 1024, 1024
ROT = H // 2            # 512 rotated channels
SA = SA_FULL // 2       # 2048 query rows per core
KC = D // 128           # 8 contraction chunks
HC = H // 128           # 8 head tiles
RC = ROT // 128         # 4 rotated head tiles
NB = 512                # free-dim chunk (= one fp32 PSUM bank)
NCH = SE // NB          # 4 column chunks
N_CORES = 8

# Partition pair-swap mask for stream_shuffle (32-partition groups).
_SWAP = [x for i in range(16) for x in (2 * i + 1, 2 * i)]

_MODULE_CACHE = None
LAST_RESULT = None


def _build_module():
    f32, f32r = mybir.dt.float32, mybir.dt.float32r
    AF = mybir.ActivationFunctionType

    nc = bacc.Bacc(trn_type="TRN2", name="xattn_head")

    xa = nc.dram_tensor("xa", [D, SA], f32r, kind="ExternalInput")   # x_audio slice, transposed
    xe = nc.dram_tensor("xe", [D, SE], f32r, kind="ExternalInput")   # x_event, transposed
    wq = nc.dram_tensor("wq", [D, H], f32r, kind="ExternalInput")    # W_q.T
    wk = nc.dram_tensor("wk", [D, H], f32r, kind="ExternalInput")    # W_k.T
    bq = nc.dram_tensor("bq", [128, HC], f32, kind="ExternalInput")  # b_q as [128, 8]
    bk = nc.dram_tensor("bk", [128, HC], f32, kind="ExternalInput")
    cq = nc.dram_tensor("cq", [ROT, SA], f32, kind="ExternalInput")  # cos, head-major
    sq = nc.dram_tensor("sq", [ROT, SA], f32, kind="ExternalInput")  # sin, sign-folded
    ck = nc.dram_tensor("ck", [ROT, SE], f32, kind="ExternalInput")
    sk = nc.dram_tensor("sk", [ROT, SE], f32, kind="ExternalInput")
    bf16 = mybir.dt.bfloat16
    mk = nc.dram_tensor("mk", [128, SE], bf16, kind="ExternalInput")  # additive mask rows
    out = nc.dram_tensor("out", [SA, SE], f32, kind="ExternalOutput")

    with TileContext(nc) as tc:
        with (
            tc.tile_pool(name="const", bufs=1) as wp,
            tc.tile_pool(name="ktp", bufs=1) as kp,
            tc.tile_pool(name="qtp", bufs=1) as qp,
            tc.tile_pool(name="xp", bufs=2) as xp,
            tc.tile_pool(name="tabp", bufs=1) as tp,
            tc.tile_pool(name="tmpp", bufs=2) as mp,
            tc.tile_pool(name="oevp", bufs=2) as op_,
            tc.tile_pool(name="pjp", bufs=4, space="PSUM") as pp,
            tc.tile_pool(name="scp", bufs=4, space="PSUM") as sp,
        ):
            wk_sb = wp.tile([128, KC * H], f32r, name="wk_sb")
            wq_sb = wp.tile([128, KC * H], f32r, name="wq_sb")
            bq_sb = wp.tile([128, HC], f32, name="bq_sb")
            bk_sb = wp.tile([128, HC], f32, name="bk_sb")
            mk_sb = wp.tile([128, SE], mybir.dt.bfloat16, name="mk_sb")

            def load_w(dst, src):
                # Per-k-chunk pieces so the pieces spread across DMA queues.
                dv = dst[:, :].rearrange("p (kc h) -> p kc h", kc=KC)
                sv = src.rearrange("(kc p) h -> p kc h", p=128)
                for k in range(KC):
                    nc.sync.dma_start(out=dv[:, k], in_=sv[:, k])

            load_w(wk_sb, wk)
            nc.sync.dma_start(out=bq_sb[:, :], in_=bq[:, :])
            nc.sync.dma_start(out=bk_sb[:, :], in_=bk[:, :])
            nc.sync.dma_start(out=mk_sb[:, :], in_=mk[:, :])

            # K^T resident in SBUF for the whole kernel: 8 tiles [128, 2048] f32r.
            kt = [kp.tile([128, SE], f32r, name=f"kt{h}", tag=f"kt{h}")
                  for h in range(HC)]

            def load_x_chunk(src, col, tag):
                x_sl = xp.tile([128, KC * NB], f32r, name="x_sl", tag=tag)
                nc.sync.dma_start(
                    out=x_sl[:, :].rearrange("p (kc n) -> p kc n", kc=KC),
                    in_=src[:, col:col + NB].rearrange("(kc p) n -> p kc n", p=128))
                return x_sl

            def load_tab(src, col, tag):
                t = tp.tile([128, RC * NB], f32, name="tab", tag=tag)
                nc.sync.dma_start(
                    out=t[:, :].rearrange("p (hc n) -> p hc n", hc=RC),
                    in_=src[:, col:col + NB].rearrange("(hc p) n -> p hc n", p=128))
                return t

            def proj_chunk(x_sl, w_sb, cos_t, sin_t, b_sb, dests):
                """One 512-column chunk of a projection: 8 head tiles, bias +
                partial rope fused into PSUM evacuation.  dests[m] = [128, NB] AP."""
                for m in range(HC):
                    ps = pp.tile([128, NB], f32, name="ps", tag="ps")
                    for k in range(KC):
                        nc.tensor.matmul(
                            ps[:, :],
                            w_sb[:, k * H + m * 128: k * H + (m + 1) * 128],
                            x_sl[:, k * NB:(k + 1) * NB],
                            start=(k == 0), stop=(k == KC - 1))
                    dst = dests[m]
                    # dst = psum + bias (ACT evacuates PSUM, per-partition bias)
                    nc.scalar.activation(dst, ps[:, :], AF.Identity,
                                         bias=b_sb[:, m:m + 1])
                    if m < RC:
                        # partial rope, in place: dst = dst*cos + pairswap(dst)*sin
                        tsh = mp.tile([128, NB], f32, name="tsh", tag="tsh")
                        nc.vector.stream_shuffle(tsh[:, :], dst, _SWAP)
                        nc.vector.tensor_mul(tsh[:, :], tsh[:, :],
                                             sin_t[:, m * NB:(m + 1) * NB])
                        nc.vector.tensor_mul(dst, dst,
                                             cos_t[:, m * NB:(m + 1) * NB])
                        nc.vector.tensor_add(dst, dst, tsh[:, :])

            # ---- Phase K: project + rope x_event into kt tiles ----
            for ec in range(NCH):
                xe_sl = load_x_chunk(xe, ec * NB, "xsl")
                ct = load_tab(ck, ec * NB, "ct")
                st = load_tab(sk, ec * NB, "st")
                proj_chunk(xe_sl, wk_sb, ct, st, bk_sb,
                           [kt[h][:, ec * NB:(ec + 1) * NB] for h in range(HC)])
                if ec == 0:
                    # Prefetch W_q.T during phase K (independent tile, no WAR).
                    load_w(wq_sb, wq)

            # ---- Phase Q + scores, fused per 512-row chunk of queries ----
            for sc in range(NCH):
                xa_sl = load_x_chunk(xa, sc * NB, "xsl")
                cqt = load_tab(cq, sc * NB, "ct")
                sqt = load_tab(sq, sc * NB, "st")
                qts = [qp.tile([128, NB], f32r, name=f"qt{h}", tag=f"qt{h}")
                       for h in range(HC)]
                proj_chunk(xa_sl, wq_sb, cqt, sqt, bq_sb, [q[:, :] for q in qts])

                for sti in range(NB // 128):
                    for ec in range(NCH):
                        ps2 = sp.tile([128, NB], f32, name="ps2", tag="ps2")
                        for h in range(HC):
                            nc.tensor.matmul(
                                ps2[:, :],
                                qts[h][:, sti * 128:(sti + 1) * 128],
                                kt[h][:, ec * NB:(ec + 1) * NB],
                                start=(h == 0), stop=(h == HC - 1))
                        osb = op_.tile([128, NB], f32, name="osb", tag="osb")
                        nc.vector.tensor_add(osb[:, :], ps2[:, :],
                                             mk_sb[:, ec * NB:(ec + 1) * NB])
                        nc.sync.dma_start(
                            out=out[sc * NB + sti * 128: sc * NB + (sti + 1) * 128,
                                    ec * NB:(ec + 1) * NB],
                            in_=osb[:, :])

    nc.compile()
    return nc


def _get_module():
    global _MODULE_CACHE
    if _MODULE_CACHE is None:
        _MODULE_CACHE = _build_module()
    return _MODULE_CACHE


def kernel(x_audio, x_event, event_padding_mask, W_q, b_q, W_k, b_k):
    global LAST_RESULT
    x_audio = np.asarray(x_audio, dtype=np.float32)
    x_event = np.asarray(x_event, dtype=np.float32)
    event_padding_mask = np.asarray(event_padding_mask)
    W_q = np.asarray(W_q, dtype=np.float32)
    b_q = np.asarray(b_q, dtype=np.float32)
    W_k = np.asarray(W_k, dtype=np.float32)
    b_k = np.asarray(b_k, dtype=np.float32)

    # Rope tables, head-major, fp32 math matching the reference.
    inv_freq = (1.0 / (10000.0 ** (np.arange(0, ROT, 2, dtype=np.float32)
                                   / np.float32(ROT)))).astype(np.float32)
    pos = np.arange(SA_FULL, dtype=np.float32)
    freqs = pos[:, None] * inv_freq[None, :]                  # [Sa, 256]
    cos_hm = np.repeat(np.cos(freqs).T, 2, axis=0)            # [512, Sa]
    sin_hm = np.repeat(np.sin(freqs).T, 2, axis=0)
    sin_hm[0::2] *= -1.0                                      # fold rotate_half sign
    cos_hm = np.ascontiguousarray(cos_hm, dtype=np.float32)
    sin_hm = np.ascontiguousarray(sin_hm, dtype=np.float32)

    wqT = np.ascontiguousarray(W_q.T)
    wkT = np.ascontiguousarray(W_k.T)
    bq_sb = np.ascontiguousarray(b_q.reshape(HC, 128).T)
    bk_sb = np.ascontiguousarray(b_k.reshape(HC, 128).T)

    import ml_dtypes
    maskadd = np.where(event_padding_mask, np.float32(0.0),
                       np.float32(-np.inf)).astype(ml_dtypes.bfloat16)  # [B, Se]
    mk_by_batch = [np.ascontiguousarray(np.broadcast_to(maskadd[b], (128, SE)))
                   for b in range(B)]
    xeT_by_batch = [np.ascontiguousarray(x_event[b].T) for b in range(B)]
    ck_hm = np.ascontiguousarray(cos_hm[:, :SE])
    sk_hm = np.ascontiguousarray(sin_hm[:, :SE])

    in_maps = []
    for c in range(N_CORES):
        b, j = divmod(c, 2)
        in_maps.append({
            "xa": np.ascontiguousarray(x_audio[b, j * SA:(j + 1) * SA].T),
            "xe": xeT_by_batch[b],
            "wq": wqT, "wk": wkT,
            "bq": bq_sb, "bk": bk_sb,
            "cq": np.ascontiguousarray(cos_hm[:, j * SA:(j + 1) * SA]),
            "sq": np.ascontiguousarray(sin_hm[:, j * SA:(j + 1) * SA]),
            "ck": ck_hm, "sk": sk_hm,
            "mk": mk_by_batch[b],
        })

    nc = _get_module()
    LAST_RESULT = run_bass_kernel_spmd(nc, in_maps, core_ids=list(range(N_CORES)))

    attn = np.empty((B, SA_FULL, SE), dtype=np.float32)
    for c in range(N_CORES):
        b, j = divmod(c, 2)
        attn[b, j * SA:(j + 1) * SA, :] = LAST_RESULT.results[c]["out"]
    return attn


# revision 12
# speedup vs baseline: 1.0350x; 1.0021x over previous
"""Trainium2 Bass kernel: cross-attention head scores with partial RoPE + padding mask.

Computes attn[b,s,e] = rope(x_audio[b] @ W_q.T + b_q)[s] . rope(x_event[b] @ W_k.T + b_k)[e]
with masked (padding) event columns set to -inf.  Shapes: B=4, Sa=4096, Se=2048,
d=1024, d_h=1024, rot_dim=512.

Sharding (8 NeuronCores): core c -> batch b = c//2, audio-row half j = c%2
(2048 query rows).  Each core computes its full [2048, 2048] score block; the
K projection is recomputed by both cores of a batch (15% extra FLOPs, no
collectives).

Per-core pipeline (all tensors head-major, i.e. Q^T/K^T as [d_h, seq]):
  1. K^T = W_k.T-proj of x_event^T on the PE (float32r, full speed), bias +
     partial rope applied on ACT/DVE during PSUM evacuation; kept in SBUF.
  2. Per 512-column chunk of Q rows: same projection + rope, then the score
     GEMM  scores[s,e] = sum_h Q^T[h,s] K^T[h,e]  on the PE, additive -inf
     mask fused into the PSUM->SBUF evacuation, DMA to DRAM.

RoPE in head-major layout: channel pairs (2i, 2i+1) live on adjacent SBUF
partitions, so rotate_half is a partition pair-swap (DVE stream_shuffle) and
the sign is folded into the sin table (host-prepared, rows 2i negated).
"""

import numpy as np

import concourse.bacc as bacc
import concourse.mybir as mybir
from concourse.tile import TileContext
from concourse.bass_utils import run_bass_kernel_spmd

# Problem constants (hardcoded per contest contract).
B, SA_FULL, SE, D, H = 4, 4096, 2048, 1024, 1024
ROT = H // 2            # 512 rotated channels
SA = SA_FULL // 2       # 2048 query rows per core
KC = D // 128           # 8 contraction chunks
HC = H // 128           # 8 head tiles
RC = ROT // 128         # 4 rotated head tiles
NB = 512                # free-dim chunk (= one fp32 PSUM bank)
NCH = SE // NB          # 4 column chunks
N_CORES = 8

# Partition pair-swap mask for stream_shuffle (32-partition groups).
_SWAP = [x for i in range(16) for x in (2 * i + 1, 2 * i)]

_MODULE_CACHE = None
LAST_RESULT = None


def _build_module():
    f32, f32r = mybir.dt.float32, mybir.dt.float32r
    AF = mybir.ActivationFunctionType

    nc = bacc.Bacc(trn_type="TRN2", name="xattn_head")

    xa = nc.dram_tensor("xa", [D, SA], f32r, kind="ExternalInput")   # x_audio slice, transposed
    xe = nc.dram_tensor("xe", [D, SE], f32r, kind="ExternalInput")   # x_event, transposed
    wq = nc.dram_tensor("wq", [D, H], f32r, kind="ExternalInput")    # W_q.T
    wk = nc.dram_tensor("wk", [D, H], f32r, kind="ExternalInput")    # W_k.T
    bq = nc.dram_tensor("bq", [128, HC], f32, kind="ExternalInput")  # b_q as [128, 8]
    bk = nc.dram_tensor("bk", [128, HC], f32, kind="ExternalInput")
    cq = nc.dram_tensor("cq", [ROT, SA], f32, kind="ExternalInput")  # cos, head-major
    sq = nc.dram_tensor("sq", [ROT, SA], f32, kind="ExternalInput")  # sin, sign-folded
    ck = nc.dram_tensor("ck", [ROT, SE], f32, kind="ExternalInput")
    sk = nc.dram_tensor("sk", [ROT, SE], f32, kind="ExternalInput")
    bf16 = mybir.dt.bfloat16
    mk = nc.dram_tensor("mk", [128, SE], bf16, kind="ExternalInput")  # additive mask rows
    out = nc.dram_tensor("out", [SA, SE], f32, kind="ExternalOutput")

    with TileContext(nc) as tc:
        with (
            tc.tile_pool(name="const", bufs=1) as wp,
            tc.tile_pool(name="ktp", bufs=1) as kp,
            tc.tile_pool(name="qtp", bufs=1) as qp,
            tc.tile_pool(name="xp", bufs=2) as xp,
            tc.tile_pool(name="tabp", bufs=1) as tp,
            tc.tile_pool(name="tmpp", bufs=2) as mp,
            tc.tile_pool(name="oevp", bufs=2) as op_,
            tc.tile_pool(name="pjp", bufs=4, space="PSUM") as pp,
            tc.tile_pool(name="scp", bufs=4, space="PSUM") as sp,
        ):
            wk_sb = wp.tile([128, KC * H], f32r, name="wk_sb")
            wq_sb = wp.tile([128, KC * H], f32r, name="wq_sb")
            bq_sb = wp.tile([128, HC], f32, name="bq_sb")
            bk_sb = wp.tile([128, HC], f32, name="bk_sb")
            mk_sb = wp.tile([128, SE], mybir.dt.bfloat16, name="mk_sb")

            def load_w(dst, src):
                # Per-(k, half) pieces: 16 transfers spread across the HWDGE
                # queues so the load runs at aggregate HBM bandwidth and the
                # k=0 pieces land early (first matmuls start sooner).
                dv = dst[:, :].rearrange("p (kc t h) -> p kc t h", kc=KC, t=2)
                sv = src.rearrange("(kc p) (t h) -> p kc t h", p=128, t=2)
                for k in range(KC):
                    for t in range(2):
                        nc.sync.dma_start(out=dv[:, k, t], in_=sv[:, k, t])

            load_w(wk_sb, wk)
            nc.gpsimd.dma_start(out=bq_sb[:, :], in_=bq[:, :])
            nc.gpsimd.dma_start(out=bk_sb[:, :], in_=bk[:, :])
            nc.gpsimd.dma_start(out=mk_sb[:, :], in_=mk[:, :])

            # K^T resident in SBUF for the whole kernel: 8 tiles [128, 2048] f32r.
            kt = [kp.tile([128, SE], f32r, name=f"kt{h}", tag=f"kt{h}")
                  for h in range(HC)]

            def load_x_chunk(src, col, tag):
                # Per-k-chunk pieces (128 KiB each) so the k=0 piece lands
                # early and the k-loop's matmuls pipeline with the DMA.
                x_sl = xp.tile([128, KC * NB], f32r, name="x_sl", tag=tag)
                dv = x_sl[:, :].rearrange("p (kc n) -> p kc n", kc=KC)
                sv = src[:, col:col + NB].rearrange("(kc p) n -> p kc n", p=128)
                for k in range(KC):
                    nc.sync.dma_start(out=dv[:, k], in_=sv[:, k])
                return x_sl

            def load_tab(src, col, tag):
                # SWDGE (gpsimd) keeps the HWDGE queues free for w/x/out.
                t = tp.tile([128, RC * NB], f32, name="tab", tag=tag)
                nc.gpsimd.dma_start(
                    out=t[:, :].rearrange("p (hc n) -> p hc n", hc=RC),
                    in_=src[:, col:col + NB].rearrange("(hc p) n -> p hc n", p=128))
                return t

            def proj_chunk(x_sl, w_sb, cos_t, sin_t, b_sb, dests):
                """One 512-column chunk of a projection: 8 head tiles, bias +
                partial rope fused into PSUM evacuation.  dests[m] = [128, NB] AP."""
                for m in range(HC):
                    ps = pp.tile([128, NB], f32, name="ps", tag="ps")
                    for k in range(KC):
                        nc.tensor.matmul(
                            ps[:, :],
                            w_sb[:, k * H + m * 128: k * H + (m + 1) * 128],
                            x_sl[:, k * NB:(k + 1) * NB],
                            start=(k == 0), stop=(k == KC - 1))
                    dst = dests[m]
                    # dst = psum + bias (ACT evacuates PSUM, per-partition bias)
                    nc.scalar.activation(dst, ps[:, :], AF.Identity,
                                         bias=b_sb[:, m:m + 1])
                    if m < RC:
                        # partial rope, in place: dst = dst*cos + pairswap(dst)*sin
                        tsh = mp.tile([128, NB], f32, name="tsh", tag="tsh")
                        nc.vector.stream_shuffle(tsh[:, :], dst, _SWAP)
                        nc.vector.tensor_mul(tsh[:, :], tsh[:, :],
                                             sin_t[:, m * NB:(m + 1) * NB])
                        nc.vector.tensor_mul(dst, dst,
                                             cos_t[:, m * NB:(m + 1) * NB])
                        nc.vector.tensor_add(dst, dst, tsh[:, :])

            # ---- Phase K: project + rope x_event into kt tiles ----
            for ec in range(NCH):
                xe_sl = load_x_chunk(xe, ec * NB, "xsl")
                ct = load_tab(ck, ec * NB, "ct")
                st = load_tab(sk, ec * NB, "st")
                proj_chunk(xe_sl, wk_sb, ct, st, bk_sb,
                           [kt[h][:, ec * NB:(ec + 1) * NB] for h in range(HC)])
                if ec == 1:
                    # Prefetch W_q.T mid phase K (independent tile, no WAR;
                    # late enough not to fight the startup DMAs for queues).
                    load_w(wq_sb, wq)

            # ---- Phase Q + scores, fused per 512-row chunk of queries ----
            for sc in range(NCH):
                xa_sl = load_x_chunk(xa, sc * NB, "xsl")
                cqt = load_tab(cq, sc * NB, "ct")
                sqt = load_tab(sq, sc * NB, "st")
                qts = [qp.tile([128, NB], f32r, name=f"qt{h}", tag=f"qt{h}")
                       for h in range(HC)]
                proj_chunk(xa_sl, wq_sb, cqt, sqt, bq_sb, [q[:, :] for q in qts])

                for sti in range(NB // 128):
                    for ecp in range(NCH // 2):
                        # Two PSUM banks per stationary Q tile: each weight
                        # load feeds the matmuls of two e-chunks.
                        pss = [sp.tile([128, NB], f32, name=f"ps2{j}", tag="ps2")
                               for j in range(2)]
                        for h in range(HC):
                            for j in range(2):
                                nc.tensor.matmul(
                                    pss[j][:, :],
                                    qts[h][:, sti * 128:(sti + 1) * 128],
                                    kt[h][:, (2 * ecp + j) * NB:(2 * ecp + j + 1) * NB],
                                    start=(h == 0), stop=(h == HC - 1))
                        for j in range(2):
                            ec = 2 * ecp + j
                            osb = op_.tile([128, NB], f32, name="osb", tag="osb")
                            nc.vector.tensor_add(osb[:, :], pss[j][:, :],
                                                 mk_sb[:, ec * NB:(ec + 1) * NB])
                            nc.sync.dma_start(
                                out=out[sc * NB + sti * 128: sc * NB + (sti + 1) * 128,
                                        ec * NB:(ec + 1) * NB],
                                in_=osb[:, :])

    nc.compile()
    return nc


def _get_module():
    global _MODULE_CACHE
    if _MODULE_CACHE is None:
        _MODULE_CACHE = _build_module()
    return _MODULE_CACHE


def kernel(x_audio, x_event, event_padding_mask, W_q, b_q, W_k, b_k):
    global LAST_RESULT
    x_audio = np.asarray(x_audio, dtype=np.float32)
    x_event = np.asarray(x_event, dtype=np.float32)
    event_padding_mask = np.asarray(event_padding_mask)
    W_q = np.asarray(W_q, dtype=np.float32)
    b_q = np.asarray(b_q, dtype=np.float32)
    W_k = np.asarray(W_k, dtype=np.float32)
    b_k = np.asarray(b_k, dtype=np.float32)

    # Rope tables, head-major, fp32 math matching the reference.
    inv_freq = (1.0 / (10000.0 ** (np.arange(0, ROT, 2, dtype=np.float32)
                                   / np.float32(ROT)))).astype(np.float32)
    pos = np.arange(SA_FULL, dtype=np.float32)
    freqs = pos[:, None] * inv_freq[None, :]                  # [Sa, 256]
    cos_hm = np.repeat(np.cos(freqs).T, 2, axis=0)            # [512, Sa]
    sin_hm = np.repeat(np.sin(freqs).T, 2, axis=0)
    sin_hm[0::2] *= -1.0                                      # fold rotate_half sign
    cos_hm = np.ascontiguousarray(cos_hm, dtype=np.float32)
    sin_hm = np.ascontiguousarray(sin_hm, dtype=np.float32)

    wqT = np.ascontiguousarray(W_q.T)
    wkT = np.ascontiguousarray(W_k.T)
    bq_sb = np.ascontiguousarray(b_q.reshape(HC, 128).T)
    bk_sb = np.ascontiguousarray(b_k.reshape(HC, 128).T)

    import ml_dtypes
    maskadd = np.where(event_padding_mask, np.float32(0.0),
                       np.float32(-np.inf)).astype(ml_dtypes.bfloat16)  # [B, Se]
    mk_by_batch = [np.ascontiguousarray(np.broadcast_to(maskadd[b], (128, SE)))
                   for b in range(B)]
    xeT_by_batch = [np.ascontiguousarray(x_event[b].T) for b in range(B)]
    ck_hm = np.ascontiguousarray(cos_hm[:, :SE])
    sk_hm = np.ascontiguousarray(sin_hm[:, :SE])

    in_maps = []
    for c in range(N_CORES):
        b, j = divmod(c, 2)
        in_maps.append({
            "xa": np.ascontiguousarray(x_audio[b, j * SA:(j + 1) * SA].T),
            "xe": xeT_by_batch[b],
            "wq": wqT, "wk": wkT,
            "bq": bq_sb, "bk": bk_sb,
            "cq": np.ascontiguousarray(cos_hm[:, j * SA:(j + 1) * SA]),
            "sq": np.ascontiguousarray(sin_hm[:, j * SA:(j + 1) * SA]),
            "ck": ck_hm, "sk": sk_hm,
            "mk": mk_by_batch[b],
        })

    nc = _get_module()
    LAST_RESULT = run_bass_kernel_spmd(nc, in_maps, core_ids=list(range(N_CORES)))

    attn = np.empty((B, SA_FULL, SE), dtype=np.float32)
    for c in range(N_CORES):
        b, j = divmod(c, 2)
        attn[b, j * SA:(j + 1) * SA, :] = LAST_RESULT.results[c]["out"]
    return attn


# revision 19
# speedup vs baseline: 1.1132x; 1.0755x over previous
"""Trainium2 Bass kernel: cross-attention head scores with partial RoPE + padding mask.

Computes attn[b,s,e] = rope(x_audio[b] @ W_q.T + b_q)[s] . rope(x_event[b] @ W_k.T + b_k)[e]
with masked (padding) event columns set to -inf.  Shapes: B=4, Sa=4096, Se=2048,
d=1024, d_h=1024, rot_dim=512.

Sharding (8 NeuronCores): core c -> batch b = c//2, audio-row half j = c%2
(2048 query rows).  Each core computes its full [2048, 2048] score block; the
K projection is recomputed by both cores of a batch (15% extra FLOPs, no
collectives).

Per-core pipeline (all tensors head-major, i.e. Q^T/K^T as [d_h, seq]):
  1. K^T = W_k.T-proj of x_event^T on the PE (float32r, full speed), bias +
     partial rope applied on ACT/DVE during PSUM evacuation; kept in SBUF.
  2. Per 512-column chunk of Q rows: same projection + rope, then the score
     GEMM  scores[s,e] = sum_h Q^T[h,s] K^T[h,e]  on the PE, additive -inf
     mask fused into the PSUM->SBUF evacuation, DMA to DRAM.

RoPE in head-major layout: channel pairs (2i, 2i+1) live on adjacent SBUF
partitions, so rotate_half is a partition pair-swap (DVE stream_shuffle) and
the sign is folded into the sin table (host-prepared, rows 2i negated).
"""

import numpy as np

import concourse.bacc as bacc
import concourse.mybir as mybir
from concourse.tile import TileContext
from concourse.bass_utils import run_bass_kernel_spmd

# Problem constants (hardcoded per contest contract).
B, SA_FULL, SE, D, H = 4, 4096, 2048, 1024, 1024
ROT = H // 2            # 512 rotated channels
SA = SA_FULL // 2       # 2048 query rows per core
KC = D // 128           # 8 contraction chunks
HC = H // 128           # 8 head tiles
RC = ROT // 128         # 4 rotated head tiles
NB = 512                # free-dim chunk (= one fp32 PSUM bank)
NCH = SE // NB          # 4 column chunks
N_CORES = 8

# Partition pair-swap mask for stream_shuffle (32-partition groups).
_SWAP = [x for i in range(16) for x in (2 * i + 1, 2 * i)]

_MODULE_CACHE = None
LAST_RESULT = None


def _build_module():
    f32, f32r = mybir.dt.float32, mybir.dt.float32r
    AF = mybir.ActivationFunctionType

    nc = bacc.Bacc(trn_type="TRN2", name="xattn_head")

    # All bulk inputs are partition-major-blocked on the host: each DMA below
    # is a straight [128, bytes] image of its SBUF destination, giving
    # fully-contiguous multi-KB packets on both sides (~2x DMA bandwidth vs
    # the naive row-major layouts).
    xa = nc.dram_tensor("xa", [NCH * 128, KC * NB], f32r, kind="ExternalInput")
    xe = nc.dram_tensor("xe", [NCH * 128, KC * NB], f32r, kind="ExternalInput")
    wq = nc.dram_tensor("wq", [128, KC * H], f32r, kind="ExternalInput")
    wk = nc.dram_tensor("wk", [128, KC * H], f32r, kind="ExternalInput")
    bq = nc.dram_tensor("bq", [128, HC], f32, kind="ExternalInput")  # b_q as [128, 8]
    bk = nc.dram_tensor("bk", [128, HC], f32, kind="ExternalInput")
    cq = nc.dram_tensor("cq", [NCH * 128, RC * NB], f32, kind="ExternalInput")
    sq = nc.dram_tensor("sq", [NCH * 128, RC * NB], f32, kind="ExternalInput")
    ck = nc.dram_tensor("ck", [NCH * 128, RC * NB], f32, kind="ExternalInput")
    sk = nc.dram_tensor("sk", [NCH * 128, RC * NB], f32, kind="ExternalInput")
    bf16 = mybir.dt.bfloat16
    mk = nc.dram_tensor("mk", [128, SE], bf16, kind="ExternalInput")  # additive mask rows
    out = nc.dram_tensor("out", [SA, SE], f32, kind="ExternalOutput")

    with TileContext(nc) as tc:
        with (
            tc.tile_pool(name="const", bufs=1) as wp,
            tc.tile_pool(name="ktp", bufs=1) as kp,
            tc.tile_pool(name="qtp", bufs=1) as qp,
            tc.tile_pool(name="xp", bufs=2) as xp,
            tc.tile_pool(name="tabp", bufs=1) as tp,
            tc.tile_pool(name="tmpp", bufs=2) as mp,
            tc.tile_pool(name="oevp", bufs=2) as op_,
            tc.tile_pool(name="pjp", bufs=4, space="PSUM") as pp,
            tc.tile_pool(name="scp", bufs=4, space="PSUM") as sp,
        ):
            wk_sb = wp.tile([128, KC * H], f32r, name="wk_sb")
            wq_sb = wp.tile([128, KC * H], f32r, name="wq_sb")
            bq_sb = wp.tile([128, HC], f32, name="bq_sb")
            bk_sb = wp.tile([128, HC], f32, name="bk_sb")
            mk_sb = wp.tile([128, SE], mybir.dt.bfloat16, name="mk_sb")

            def load_w(dst, src):
                nc.sync.dma_start(out=dst[:, :], in_=src[:, :])

            load_w(wk_sb, wk)
            nc.gpsimd.dma_start(out=bq_sb[:, :], in_=bq[:, :])
            nc.gpsimd.dma_start(out=bk_sb[:, :], in_=bk[:, :])
            nc.gpsimd.dma_start(out=mk_sb[:, :], in_=mk[:, :])

            # K^T resident in SBUF for the whole kernel: 8 tiles [128, 2048] f32r.
            kt = [kp.tile([128, SE], f32r, name=f"kt{h}", tag=f"kt{h}")
                  for h in range(HC)]

            def load_x_chunk(src, chunk, tag):
                x_sl = xp.tile([128, KC * NB], f32r, name="x_sl", tag=tag)
                nc.sync.dma_start(out=x_sl[:, :],
                                  in_=src[chunk * 128:(chunk + 1) * 128, :])
                return x_sl

            def load_tab(src, chunk, tag):
                t = tp.tile([128, RC * NB], f32, name="tab", tag=tag)
                nc.sync.dma_start(out=t[:, :],
                                  in_=src[chunk * 128:(chunk + 1) * 128, :])
                return t

            def proj_chunk(x_sl, w_sb, cos_t, sin_t, b_sb, dests):
                """One 512-column chunk of a projection: 8 head tiles, bias +
                partial rope fused into PSUM evacuation.  dests[m] = [128, NB] AP."""
                for m in range(HC):
                    ps = pp.tile([128, NB], f32, name="ps", tag="ps")
                    for k in range(KC):
                        nc.tensor.matmul(
                            ps[:, :],
                            w_sb[:, k * H + m * 128: k * H + (m + 1) * 128],
                            x_sl[:, k * NB:(k + 1) * NB],
                            start=(k == 0), stop=(k == KC - 1))
                    dst = dests[m]
                    # dst = psum + bias (ACT evacuates PSUM, per-partition bias)
                    nc.scalar.activation(dst, ps[:, :], AF.Identity,
                                         bias=b_sb[:, m:m + 1])
                    if m < RC:
                        # partial rope, in place: dst = dst*cos + pairswap(dst)*sin
                        tsh = mp.tile([128, NB], f32, name="tsh", tag="tsh")
                        nc.vector.stream_shuffle(tsh[:, :], dst, _SWAP)
                        nc.vector.tensor_mul(tsh[:, :], tsh[:, :],
                                             sin_t[:, m * NB:(m + 1) * NB])
                        nc.vector.tensor_mul(dst, dst,
                                             cos_t[:, m * NB:(m + 1) * NB])
                        nc.vector.tensor_add(dst, dst, tsh[:, :])

            # ---- Phase K: project + rope x_event into kt tiles ----
            for ec in range(NCH):
                xe_sl = load_x_chunk(xe, ec, "xsl")
                ct = load_tab(ck, ec, "ct")
                st = load_tab(sk, ec, "st")
                proj_chunk(xe_sl, wk_sb, ct, st, bk_sb,
                           [kt[h][:, ec * NB:(ec + 1) * NB] for h in range(HC)])
                if ec == 1:
                    # Prefetch W_q.T mid phase K (independent tile, no WAR;
                    # late enough not to fight the startup DMAs for queues).
                    load_w(wq_sb, wq)

            # ---- Phase Q + scores, fused per 512-row chunk of queries ----
            for sc in range(NCH):
                xa_sl = load_x_chunk(xa, sc, "xsl")
                cqt = load_tab(cq, sc, "ct")
                sqt = load_tab(sq, sc, "st")
                qts = [qp.tile([128, NB], f32r, name=f"qt{h}", tag=f"qt{h}")
                       for h in range(HC)]
                proj_chunk(xa_sl, wq_sb, cqt, sqt, bq_sb, [q[:, :] for q in qts])

                for sti in range(NB // 128):
                    for ecp in range(NCH // 2):
                        # Two PSUM banks per stationary Q tile: each weight
                        # load feeds the matmuls of two e-chunks.
                        pss = [sp.tile([128, NB], f32, name=f"ps2{j}", tag="ps2")
                               for j in range(2)]
                        for h in range(HC):
                            for j in range(2):
                                nc.tensor.matmul(
                                    pss[j][:, :],
                                    qts[h][:, sti * 128:(sti + 1) * 128],
                                    kt[h][:, (2 * ecp + j) * NB:(2 * ecp + j + 1) * NB],
                                    start=(h == 0), stop=(h == HC - 1))
                        for j in range(2):
                            ec = 2 * ecp + j
                            osb = op_.tile([128, NB], f32, name="osb", tag="osb")
                            nc.vector.tensor_add(osb[:, :], pss[j][:, :],
                                                 mk_sb[:, ec * NB:(ec + 1) * NB])
                            nc.sync.dma_start(
                                out=out[sc * NB + sti * 128: sc * NB + (sti + 1) * 128,
                                        ec * NB:(ec + 1) * NB],
                                in_=osb[:, :])

    nc.compile()
    return nc


def _get_module():
    global _MODULE_CACHE
    if _MODULE_CACHE is None:
        _MODULE_CACHE = _build_module()
    return _MODULE_CACHE


def kernel(x_audio, x_event, event_padding_mask, W_q, b_q, W_k, b_k):
    global LAST_RESULT
    x_audio = np.asarray(x_audio, dtype=np.float32)
    x_event = np.asarray(x_event, dtype=np.float32)
    event_padding_mask = np.asarray(event_padding_mask)
    W_q = np.asarray(W_q, dtype=np.float32)
    b_q = np.asarray(b_q, dtype=np.float32)
    W_k = np.asarray(W_k, dtype=np.float32)
    b_k = np.asarray(b_k, dtype=np.float32)

    # Rope tables, head-major, fp32 math matching the reference.
    inv_freq = (1.0 / (10000.0 ** (np.arange(0, ROT, 2, dtype=np.float32)
                                   / np.float32(ROT)))).astype(np.float32)
    pos = np.arange(SA_FULL, dtype=np.float32)
    freqs = pos[:, None] * inv_freq[None, :]                  # [Sa, 256]
    cos_hm = np.repeat(np.cos(freqs).T, 2, axis=0)            # [512, Sa]
    sin_hm = np.repeat(np.sin(freqs).T, 2, axis=0)
    sin_hm[0::2] *= -1.0                                      # fold rotate_half sign
    cos_hm = np.ascontiguousarray(cos_hm, dtype=np.float32)
    sin_hm = np.ascontiguousarray(sin_hm, dtype=np.float32)

    def block_w(wT):
        # [D, H] -> [128, KC*H]: partition-major image of the SBUF tile.
        return np.ascontiguousarray(
            wT.reshape(KC, 128, H).transpose(1, 0, 2).reshape(128, KC * H))

    def block_x(xT):
        # [D, S] -> [NCH*128, KC*NB]: per-column-chunk SBUF images.
        return np.ascontiguousarray(
            xT.reshape(KC, 128, NCH, NB).transpose(2, 1, 0, 3)
              .reshape(NCH * 128, KC * NB))

    def block_tab(t):
        # [ROT, S] -> [NCH*128, RC*NB]: per-column-chunk SBUF images.
        return np.ascontiguousarray(
            t.reshape(RC, 128, NCH, NB).transpose(2, 1, 0, 3)
             .reshape(NCH * 128, RC * NB))

    wqT = block_w(W_q.T)
    wkT = block_w(W_k.T)
    bq_sb = np.ascontiguousarray(b_q.reshape(HC, 128).T)
    bk_sb = np.ascontiguousarray(b_k.reshape(HC, 128).T)

    import ml_dtypes
    maskadd = np.where(event_padding_mask, np.float32(0.0),
                       np.float32(-np.inf)).astype(ml_dtypes.bfloat16)  # [B, Se]
    mk_by_batch = [np.ascontiguousarray(np.broadcast_to(maskadd[b], (128, SE)))
                   for b in range(B)]
    xeT_by_batch = [block_x(x_event[b].T) for b in range(B)]
    ck_hm = block_tab(cos_hm[:, :SE])
    sk_hm = block_tab(sin_hm[:, :SE])
    cq_half = [block_tab(cos_hm[:, j * SA:(j + 1) * SA]) for j in range(2)]
    sq_half = [block_tab(sin_hm[:, j * SA:(j + 1) * SA]) for j in range(2)]

    in_maps = []
    for c in range(N_CORES):
        b, j = divmod(c, 2)
        in_maps.append({
            "xa": block_x(x_audio[b, j * SA:(j + 1) * SA].T),
            "xe": xeT_by_batch[b],
            "wq": wqT, "wk": wkT,
            "bq": bq_sb, "bk": bk_sb,
            "cq": cq_half[j], "sq": sq_half[j],
            "ck": ck_hm, "sk": sk_hm,
            "mk": mk_by_batch[b],
        })

    nc = _get_module()
    LAST_RESULT = run_bass_kernel_spmd(nc, in_maps, core_ids=list(range(N_CORES)))

    attn = np.empty((B, SA_FULL, SE), dtype=np.float32)
    for c in range(N_CORES):
        b, j = divmod(c, 2)
        attn[b, j * SA:(j + 1) * SA, :] = LAST_RESULT.results[c]["out"]
    return attn
